# revision 2
# baseline (speedup 1.0000x reference)
"""KMeans soft-assignment layer (vq_codebook) for 8x TRN2 NeuronCores.

softmax(-||x-c||^2 / T) over K=512 centroids, T=0.1.

Math: softmax is invariant to the per-row ||x||^2 term, so
logits = (2*x.c - ||c||^2) / T = x @ (20*c)^T - 10*||c||^2.
The -10*||c||^2 row rides the matmul as an extra contraction row against a
ones-row appended to x^T (lhsT = [x^T; 1], rhs = [20*c^T; -10*csq]).

Sharding: data-parallel, batch b -> core b. Each core: 32768 tokens.
Per-core per 128-token tile:
  PE   : float32r matmul [65,128]^T @ [65,512] -> PSUM logits [128,512]
         (fp32r = 1 cycle/row on TRN2 PE, ~2^-13 logit error - well within
         the 2e-2 gate; verified end-to-end rel err ~5e-3)
  DVE  : reduce_max(negate) -> -m [128,1]
  ACT  : exp(logits - m) with fused row-sum accum -> e [128,512], s
  DVE  : one reciprocal per 4-tile group on s [128,4] -> r
  DVE  : e * r -> out tile
  DMA  : out tile -> HBM (sync ring; input DMAs ride the ACT ring)
"""
import sys

sys.path.insert(0, "/opt/trn_rl_repo")

from contextlib import ExitStack

import numpy as np
import ml_dtypes

import concourse.bacc as bacc
import concourse.bass as bass
import concourse.mybir as mybir
import concourse.tile as tile
from concourse.bass_utils import run_bass_kernel_spmd

N_CORES = 8
B, S, D = 8, 32768, 64
K = 512
TEMP = 0.1
P = 128
N_TILES = S // P
CD = D + 1

F32 = mybir.dt.float32
F32R = mybir.dt.float32r
BF16 = mybir.dt.bfloat16

_NC_CACHE = {}
BEST = dict(mm="f32r", norm_pattern="D", bufs_ps=6, bufs_e=8, bufs_o=6,
            chunk=16, group=4, in_dma="scalar")


def _build_nc(
    repeats=1,
    mm="f32r",
    bufs_in=3,
    bufs_ps=6,
    bufs_e=8,
    bufs_o=6,
    chunk=16,
    group=4,
    norm_pattern="D",
    in_dma="scalar",
):
    nc = bacc.Bacc(
        "TRN2", target_bir_lowering=False, debug=False, num_devices=N_CORES
    )
    out = nc.declare_dram_parameter("out", [S, K], F32, isOutput=True)
    in_eng = nc.sync if in_dma == "sync" else nc.scalar

    with tile.TileContext(nc) as tc, ExitStack() as ctx:
        const_pool = ctx.enter_context(tc.tile_pool(name="const", bufs=1))
        in_pool = ctx.enter_context(tc.tile_pool(name="xin", bufs=bufs_in))
        psum_pool = ctx.enter_context(
            tc.tile_pool(name="ps", bufs=bufs_ps, space="PSUM")
        )
        e_pool = ctx.enter_context(tc.tile_pool(name="e", bufs=bufs_e))
        o_pool = ctx.enter_context(tc.tile_pool(name="o", bufs=bufs_o))
        stat_pool = ctx.enter_context(tc.tile_pool(name="stat", bufs=12))
        sg_pool = ctx.enter_context(tc.tile_pool(name="sg", bufs=3))

        if mm == "bf3":
            xth = nc.declare_dram_parameter("xth", [CD, S], BF16,
                                            isOutput=False)
            xtl = nc.declare_dram_parameter("xtl", [CD, S], BF16,
                                            isOutput=False)
            rhh = nc.declare_dram_parameter("rhh", [CD, K], BF16,
                                            isOutput=False)
            rhl = nc.declare_dram_parameter("rhl", [CD, K], BF16,
                                            isOutput=False)
            rhs_h = const_pool.tile([CD, K], BF16)
            rhs_l = const_pool.tile([CD, K], BF16)
            nc.sync.dma_start(rhs_h[:], rhh[:])
            nc.sync.dma_start(rhs_l[:], rhl[:])
        else:
            xt = nc.declare_dram_parameter("xt", [CD, S], F32R,
                                           isOutput=False)
            rh = nc.declare_dram_parameter("rh", [CD, K], F32R,
                                           isOutput=False)
            rhs = const_pool.tile([CD, K], F32R)
            nc.sync.dma_start(rhs[:], rh[:])

        for _rep in range(repeats):
            for c in range(N_TILES // chunk):
                cs = c * P * chunk
                ce = (c + 1) * P * chunk
                if mm == "bf3":
                    xin_h = in_pool.tile([CD, P * chunk], BF16, tag="xh")
                    xin_l = in_pool.tile([CD, P * chunk], BF16, tag="xl")
                    in_eng.dma_start(xin_h[:], xth[:, cs:ce])
                    in_eng.dma_start(xin_l[:], xtl[:, cs:ce])
                else:
                    xin = in_pool.tile([CD, P * chunk], F32R, tag="x")
                    in_eng.dma_start(xin[:], xt[:, cs:ce])
                for j in range(chunk):
                    t = c * chunk + j
                    g = t % group
                    if g == 0:
                        s_g = sg_pool.tile([P, group], F32, tag="s")
                        r_g = sg_pool.tile([P, group], F32, tag="r")
                        es = []
                    ps = psum_pool.tile([P, K], F32)
                    if mm == "bf3":
                        xh = xin_h[:, j * P : (j + 1) * P]
                        xl = xin_l[:, j * P : (j + 1) * P]
                        nc.tensor.matmul(ps[:], xh, rhs_h[:], start=True,
                                         stop=False)
                        nc.tensor.matmul(ps[:], xh, rhs_l[:], start=False,
                                         stop=False)
                        nc.tensor.matmul(ps[:], xl, rhs_h[:], start=False,
                                         stop=True)
                    else:
                        nc.tensor.matmul(ps[:], xin[:, j * P : (j + 1) * P],
                                         rhs[:], start=True, stop=True)
                    nm = stat_pool.tile([P, 1], F32)
                    nc.vector.tensor_reduce(
                        nm[:], ps[:],
                        axis=mybir.AxisListType.X, op=mybir.AluOpType.max,
                        negate=True,
                    )
                    e = e_pool.tile([P, K], F32)
                    nc.scalar.activation(
                        e[:], ps[:], mybir.ActivationFunctionType.Exp,
                        bias=nm[:], scale=1.0, accum_out=s_g[:, g : g + 1],
                    )
                    es.append(e)
                    if g == group - 1:
                        nc.vector.reciprocal(r_g[:], s_g[:])
                        for gg, eg in enumerate(es):
                            tt = t - (group - 1) + gg
                            o = o_pool.tile([P, K], F32)
                            eng = norm_pattern[tt % len(norm_pattern)]
                            r_ap = r_g[:, gg : gg + 1]
                            if eng == "G":
                                nc.gpsimd.tensor_scalar_mul(o[:], eg[:], r_ap)
                            elif eng == "D":
                                nc.vector.tensor_scalar_mul(o[:], eg[:], r_ap)
                            else:
                                nc.scalar.activation(
                                    o[:], eg[:],
                                    mybir.ActivationFunctionType.Copy,
                                    scale=r_ap,
                                )
                            nc.sync.dma_start(
                                out[tt * P : (tt + 1) * P, :], o[:]
                            )
    nc.compile()
    return nc


def _prep_inputs(x, centroids, mm="f32r"):
    c64 = centroids.astype(np.float64)
    csq = np.sum(c64**2, axis=1)
    rh64 = np.empty((CD, K), np.float64)
    rh64[0:D] = (2.0 / TEMP) * c64.T
    rh64[D] = -csq / TEMP
    in_maps = []
    if mm == "bf3":
        rhh = rh64.astype(ml_dtypes.bfloat16)
        rhl = (rh64 - rhh.astype(np.float64)).astype(ml_dtypes.bfloat16)
        for b in range(N_CORES):
            xt = x[b].T.astype(np.float64)
            xth = np.empty((CD, S), ml_dtypes.bfloat16)
            xtl = np.empty((CD, S), ml_dtypes.bfloat16)
            xth[0:D] = xt.astype(ml_dtypes.bfloat16)
            xtl[0:D] = (xt - xth[0:D].astype(np.float64)).astype(
                ml_dtypes.bfloat16)
            xth[D] = 1.0
            xtl[D] = 0.0
            in_maps.append({
                "xth": np.ascontiguousarray(xth),
                "xtl": np.ascontiguousarray(xtl),
                "rhh": rhh, "rhl": rhl,
            })
    else:
        rh = rh64.astype(np.float32)
        for b in range(N_CORES):
            xt = np.empty((CD, S), np.float32)
            xt[0:D] = x[b].T
            xt[D] = 1.0
            in_maps.append({"xt": np.ascontiguousarray(xt), "rh": rh})
    return in_maps


def kernel(x, centroids):
    x = np.asarray(x)
    centroids = np.asarray(centroids)
    in_maps = _prep_inputs(x, centroids, BEST["mm"])

    if "nc" not in _NC_CACHE:
        _NC_CACHE["nc"] = _build_nc(1, **BEST)
    nc = _NC_CACHE["nc"]

    res = run_bass_kernel_spmd(nc, in_maps, list(range(N_CORES))).results
    out = np.stack([res[b]["out"] for b in range(N_CORES)], axis=0)
    return out.reshape(B, S, K)


if __name__ == "__main__":
    xs = np.random.randn(B, S, D).astype(np.float32)
    cs = np.random.randn(K, D).astype(np.float32)
    o = kernel(xs, cs)
    print(o.shape, o.dtype, o[0, 0, :4])
